# revision 80
# baseline (speedup 1.0000x reference)
"""Trainium2 Bass kernel for nn_DFNPureModel (retrieval_knn).

Data-parallel over batch B=8 across 8 NeuronCores; params replicated.
Per core (one batch element, S=4096 tokens, D=512, F=512, E=1024, Ne=512,
G2=10000 grid points):

  chain (f32r, feature-major):  h1 = gelu(x@W1), df = gelu(h1@W2),
           g = gelu(df@We1eff)  with  We1eff = We1[2:] + Wc@We1[:2].
           importance^2 = g^T A g + v^T g with A = We2 We2^T (host).
           In-chain: g is copied to bf16 (Act), PE-transposed to token-major
           and spilled to DRAM so selection gathers with no post-chain pass.
  top-k:   kth_largest (GPSIMD) -> mask+iota+sparse_gather -> token idx;
           dma_gather (transpose) -> g_sel feature-major bf16.
  pass 2:  ef_sel = g_sel@We2r: We2 columns reordered on host so the states
           land 128-aligned (wo1 contracts exactly 8 chunks) and the
           positions form a tiny separate group computed FIRST.
  attention (the big rewrite): positions cluster near the origin, so with
           X = exp(20 px g + c), Y = exp(20 py g), X' = X-1, Y' = Y-1:
             e[n,g] = X[n,ix]Y[n,iy],   e = 1 + X'Y + Y'
             out*den = P0 + (X'Y)^T proj + W2[iy(g),:],  W2 = Y'^T proj
           P0 = colsum(proj) rides the wo2 bias-add (accum_out, exact f32).
           The (X'Y)-field quantizes to fp8e4m3 at x16 (errors scale with
           |X'Y| ~ 0.1, so fp8 beats the old bf16-e by ~5x) and the big GEMM
           runs MatmulPerfMode.DoubleRow (2 k-tiles per matmul).  The W2
           term enters the SAME fp8 GEMM through two constant indicator
           k-tiles (values 32/8) against a hi/lo fp8 split of W2 - one
           extra DoubleRow matmul per tile, no elementwise work.
           The e-field fill is fp8(16*X' (*) Y) per 500-grid chunk:
           fused 3-D-broadcast STTs on DVE (13 chunks) and tensor_mul +
           Act-quant pairs on Pool (7 chunks), no PE.
           den = X^T Y + corr: one rank-100 GEMM; rec = 1/den replicated
           across partitions by a stride-0 DMA roundtrip; the combine is
           (psum + 65536*P0) * rec/65536 -> bf16: a fused STT on DVE for
           12/20 tiles, an Act Identity psum-drain + Pool tensor_mul for
           the rest (Pool cannot read PSUM).
           Output is written transposed [D, G] in bf16 (host does .T).
  overlap: e-fill runs under wo1; wo2 and attn interleave per dim-chunk so
           out DMA (alternating queues) starts early; PE prewarm covers the
           initial x/w1 DMA; a warm-keeper bridges the selection gap, and a
           dummy idxr-dependent DMA defers wo1's transfers so the gather
           wins the FIFO DMA-engine arbitration.

Known TRN2 hazards handled: f32r matmul needs rounded producers and crashes
for 1<M<128 (only M=1/M=128 used); memset cannot write f32r; partition-dim
stride-0 APs are rejected by engines (the rec replicate rides a DMA, which
allows them on the DRAM side); walrus rejects STT/psum-reads on Pool, 4-D
STT access patterns, engine partition bases off 0/32/64/96, and accum_out
without an explicit op1; DMA waits hold the issuing sequencer, so dependent
DMAs alternate queues.
"""

import numpy as np
import ml_dtypes

import concourse.bass as bass
import concourse.mybir as mybir
import concourse.tile as tile
from concourse import bacc
from concourse.bass_utils import run_bass_kernel_spmd
from concourse.masks import make_identity

F32 = mybir.dt.float32
F32R = mybir.dt.float32r
BF16 = mybir.dt.bfloat16
F8 = mybir.dt.float8e4
I16 = mybir.dt.int16
U32 = mybir.dt.uint32
AF = mybir.ActivationFunctionType
ALU = mybir.AluOpType
DR = mybir.MatmulPerfMode.DoubleRow

B, S, D, F, E, NE = 8, 4096, 512, 512, 1024, 512
EFR = 1026          # reordered We2 cols: 1024 states + 2 pos
G2 = 10000
GP = 10240          # padded grid (20 chunks of 512)
TB = 8              # token blocks
TT = 512            # tokens per block

DSC = 16.0          # fp8 scale on the X'Y field
PSC = 4096.0        # fp8 scale on proj
OSC = DSC * PSC     # psum scale
CHAIN = "f32r"


def _build(chain=CHAIN):
    nc = bacc.Bacc("TRN2", target_bir_lowering=False, debug=False,
                   enable_asserts=True, num_devices=8)

    def din(name, shape, dt):
        return nc.dram_tensor(name, list(shape), dt, kind="ExternalInput").ap()

    x_d = din("xT", [D, S], F32R)       # host-transposed x
    a_d = din("aq", [128, 4, F], F32R)  # A = We2 We2^T (importance quadratic)
    v_d = din("vq", [128, 4], F32)      # v = 2 We2 be2
    w1_d = din("w1", [128, 4, F], F32R)
    w2_d = din("w2", [128, 4, F], F32R)
    we1_d = din("we1", [128, 4, F], F32R)   # We1[2:] + Wc@We1[:2]
    we2_d = din("we2r", [128, 4, EFR], BF16)  # reordered: states | pos
    wo1_d = din("wo1", [128, 8, 2 * E], BF16)  # unpadded, states-aligned
    wo2_d = din("wo2", [128, 16, D], BF16)
    b1_d = din("b1", [128, 4], F32)
    b2_d = din("b2", [128, 4], F32)
    be1_d = din("be1", [128, 4], F32)   # be1 + bc @ We1[:2]
    be2_d = din("be2r", [128, 9], F32)  # chunks 0-7 states bias; col 8 pos
    bo1_d = din("bo1", [128, 16], F32)
    bo2_d = din("bo2", [128, 4], F32)
    gv_d = din("gv", [128, 100], F32)   # grid values replicated
    corr_d = din("corrM", [100, 100], F32)  # 1e-8*exp(10(gx^2+gy^2))
    ind_d = din("ind", [128, 2, GP], F8)    # 32/8 iy-indicator k-tiles
    out_d = nc.dram_tensor("out", [D, GP], BF16, kind="ExternalOutput").ap()
    # token-major bf16 spill of g for the selection dma_gather
    gtm_d = nc.dram_tensor("gtm", [S, F], BF16, kind="Internal").ap()
    rec_d = nc.dram_tensor("recd", [GP], F32, kind="Internal").ap()
    scr_d = nc.dram_tensor("scrd", [128, 32], I16, kind="Internal").ap()

    with tile.TileContext(nc) as tc:
        with tc.tile_pool(name="small", bufs=1) as small:
            b1 = small.tile([128, 4], F32)
            b2 = small.tile([128, 4], F32)
            be1 = small.tile([128, 4], F32)
            be2 = small.tile([128, 9], F32)
            bo1 = small.tile([128, 16], F32)
            bo2 = small.tile([128, 4], F32)
            gv = small.tile([128, 100], F32)
            corrM = small.tile([100, 100], F32)
            ident = small.tile([128, 128], F32)
            make_identity(nc, ident[:])
            ones_f32 = small.tile([128, 1], F32)
            nc.vector.memset(ones_f32[:], 1.0)
            ones_col = small.tile([128, 1], F32R)
            nc.vector.tensor_copy(ones_col[:], ones_f32[:])
            vq = small.tile([128, 4], F32)
            # importance^2 in both selection layouts, filled per token block
            ipm = small.tile([128, 32], F32)
            iiv = small.tile([16, 256], F32)
            ident_bf = small.tile([128, 128], BF16)
            nc.vector.tensor_copy(ident_bf[:], ident[:])
            repmat = small.tile([16, 128], F32)
            for c in range(8):
                nc.vector.tensor_copy(repmat[:, c * 16:(c + 1) * 16],
                                      ident[0:16, 0:16])

            with tc.tile_pool(name="ob2", bufs=1) as ob2:
                # output-side weights, transferred during the chain (also
                # feeds the selection-window warm-keeper matmuls)
                wo2 = ob2.tile([128, 16, D], BF16)

                # ============ chain ============
                with tc.tile_pool(name="gbuf", bufs=1) as gbuf, \
                     tc.tile_pool(name="wts", bufs=1) as wts, \
                     tc.tile_pool(name="pa", bufs=2) as pa, \
                     tc.tile_pool(name="pa1", bufs=1) as pa1, \
                     tc.tile_pool(name="spl", bufs=1) as spl, \
                     tc.tile_pool(name="mm_ps", bufs=3, space="PSUM") as mm_ps, \
                     tc.tile_pool(name="imp_ps", bufs=1, space="PSUM") as imp_ps, \
                     tc.tile_pool(name="tp_ps", bufs=2, space="PSUM") as tp_ps:
                    gT = gbuf.tile([128, 4, S], F32R)
                    w1 = wts.tile([128, 4, F], F32R)
                    w2 = wts.tile([128, 4, F], F32R)
                    we1 = wts.tile([128, 4, F], F32R)
                    aq = wts.tile([128, 4, F], F32R)

                    xts = {}

                    def load_x(tb):
                        t = pa.tile([128, 4, TT], F32R, tag="xT")
                        nc.sync.dma_start(
                            t[:], x_d.rearrange("(c p) t -> p c t", p=128)
                            [:, :, tb * TT:(tb + 1) * TT])
                        xts[tb] = t

                    # every transfer the chain depends on rides ONE queue in
                    # priority order (DMA engines serve a single transfer at
                    # a time, in acquisition order).
                    xt0 = pa.tile([128, 4, TT], F32R, tag="xT")
                    xr = x_d.rearrange("(c p) t -> p c t", p=128)
                    for kh in range(2):
                        ks = slice(2 * kh, 2 * kh + 2)
                        nc.sync.dma_start(w1[:, ks, :], w1_d[:, ks, :])
                        nc.sync.dma_start(xt0[:, ks, :], xr[:, ks, 0:TT])
                    xts[0] = xt0
                    nc.sync.dma_start(b1[:], b1_d[:])
                    nc.sync.dma_start(b2[:], b2_d[:])
                    for kh in range(2):
                        ks = slice(2 * kh, 2 * kh + 2)
                        nc.sync.dma_start(w2[:, ks, :], w2_d[:, ks, :])
                    nc.sync.dma_start(be1[:], be1_d[:])
                    for kh in range(2):
                        ks = slice(2 * kh, 2 * kh + 2)
                        nc.sync.dma_start(we1[:, ks, :], we1_d[:, ks, :])
                    for kh in range(2):
                        ks = slice(2 * kh, 2 * kh + 2)
                        nc.sync.dma_start(aq[:, ks, :], a_d[:, ks, :])
                    load_x(1)
                    nc.sync.dma_start(vq[:], v_d[:])
                    nc.sync.dma_start(be2[:], be2_d[:])
                    nc.scalar.dma_start(wo2[:, 0:8, :], wo2_d[:, 0:8, :])
                    nc.scalar.dma_start(wo2[:, 8:16, :], wo2_d[:, 8:16, :])
                    nc.scalar.dma_start(gv[:], gv_d[:])
                    nc.scalar.dma_start(corrM[:], corr_d[:])
                    nc.scalar.dma_start(bo1[:], bo1_d[:])
                    nc.scalar.dma_start(bo2[:], bo2_d[:])

                    # prewarm: keep PE busy from ~t=0.3us so the chain starts
                    # at a ramped clock instead of pstate-low
                    for wi in range(3):
                        wt = mm_ps.tile([128, 128], F32, tag="mm", name="wt")
                        nc.tensor.matmul(wt[:], ident[:], ident[:],
                                         start=True, stop=True)

                    for tb in range(TB):
                        tok = slice(tb * TT, (tb + 1) * TT)
                        xT = xts.pop(tb)
                        if tb + 1 < TB and tb > 0:
                            load_x(tb + 1)

                        h1g = pa.tile([128, 4, TT], F32R, tag="h1g")
                        if tb == 0:
                            # k-outer in two m-halves: matmuls start when the
                            # first w1/x0 chunk lands
                            for half in range(2):
                                pss = [mm_ps.tile([128, TT], F32,
                                                  tag=f"mm0{m}", bufs=1,
                                                  name=f"ps0{m}")
                                       for m in range(2)]
                                for k in range(4):
                                    for m2 in range(2):
                                        m = half * 2 + m2
                                        nc.tensor.matmul(
                                            pss[m2][:],
                                            w1[:, k, m * 128:(m + 1) * 128],
                                            xT[:, k, :], start=(k == 0),
                                            stop=(k == 3))
                                for m2 in range(2):
                                    m = half * 2 + m2
                                    nc.scalar.activation(
                                        h1g[:, m, :], pss[m2][:], AF.Gelu,
                                        bias=b1[:, m:m + 1])
                        else:
                            for m in range(4):
                                ps = mm_ps.tile([128, TT], F32, tag="mm")
                                for k in range(4):
                                    nc.tensor.matmul(
                                        ps[:], w1[:, k, m * 128:(m + 1) * 128],
                                        xT[:, k, :], start=(k == 0),
                                        stop=(k == 3))
                                nc.scalar.activation(h1g[:, m, :], ps[:],
                                                     AF.Gelu,
                                                     bias=b1[:, m:m + 1])

                        dfg = pa.tile([128, 4, TT], F32R, tag="dfg")
                        for m in range(4):
                            ps = mm_ps.tile([128, TT], F32, tag="mm")
                            for k in range(4):
                                nc.tensor.matmul(
                                    ps[:], w2[:, k, m * 128:(m + 1) * 128],
                                    h1g[:, k, :], start=(k == 0), stop=(k == 3))
                            nc.scalar.activation(dfg[:, m, :], ps[:], AF.Gelu,
                                                 bias=b2[:, m:m + 1])

                        gbf = spl.tile([128, 4, TT], BF16, tag="gbf", bufs=2)
                        for m in range(4):
                            ps = mm_ps.tile([128, TT], F32, tag="mm")
                            for k in range(4):
                                nc.tensor.matmul(
                                    ps[:], we1[:, k, m * 128:(m + 1) * 128],
                                    dfg[:, k, :], start=(k == 0), stop=(k == 3))
                            nc.scalar.activation(gT[:, m, tok], ps[:], AF.Gelu,
                                                 bias=be1[:, m:m + 1])
                            nc.scalar.activation(gbf[:, m, :],
                                                 gT[:, m, tok].bitcast(F32),
                                                 AF.Copy)

                        def imp_pass():
                            # importance^2 = g^T A g + v^T g; the elementwise
                            # (Ag+v)*g splits DVE/Pool so the A-GEMM's psum
                            # rotation never throttles on one engine; psi
                            # accumulates the two halves
                            psi = imp_ps.tile([1, TT], F32, tag="psi",
                                              name="psi")
                            accs = []
                            for h in range(2):
                                acc = pa.tile([128, TT], F32R,
                                              tag=f"prod{h}", name="acc")
                                accs.append(acc)
                                for m2 in range(2):
                                    m = 2 * h + m2
                                    ps = mm_ps.tile([128, TT], F32, tag="mm",
                                                    name="ps")
                                    for k in range(4):
                                        nc.tensor.matmul(
                                            ps[:],
                                            aq[:, k, m * 128:(m + 1) * 128],
                                            gT[:, k, tok],
                                            start=(k == 0), stop=(k == 3))
                                    # Pool can't read PSUM (walrus), so both
                                    # halves run the fused STT on DVE
                                    dst = acc
                                    if m2 == 1:
                                        dst = pa.tile([128, TT], F32R,
                                                      tag=f"prodm{h}",
                                                      name="prod")
                                    nc.vector.scalar_tensor_tensor(
                                        dst[:], ps[:], vq[:, m:m + 1],
                                        gT[:, m, tok].bitcast(F32),
                                        op0=ALU.add, op1=ALU.mult)
                                    if m2 == 1:
                                        nc.vector.tensor_add(
                                            acc[:], acc[:].bitcast(F32),
                                            dst[:].bitcast(F32))
                            for h in range(2):
                                nc.tensor.matmul(psi[:], ones_col[:],
                                                 accs[h][:], start=(h == 0),
                                                 stop=(h == 1))
                            imp_tb = pa1.tile([1, TT], F32, tag="imp_tb",
                                              bufs=2, name="imp_tb")
                            nc.scalar.copy(imp_tb[:], psi[:])
                            nc.sync.dma_start(
                                ipm[16 * tb:16 * (tb + 1), :],
                                imp_tb[0:1, :].rearrange("p (a b) -> p a b",
                                                         a=16))
                            nc.sync.dma_start(iiv[2 * tb:2 * (tb + 1), :],
                                              imp_tb[0:1, :])

                        def spill_pass():
                            # in-chain token-major bf16 spill (PE transposes)
                            for ti in range(4):
                                tpp = tp_ps.tile([128, TT], BF16, tag="tp",
                                                 name="tpp")
                                for k in range(4):
                                    nc.tensor.transpose(
                                        tpp[:, k * 128:(k + 1) * 128],
                                        gbf[:, k, ti * 128:(ti + 1) * 128],
                                        ident_bf[:])
                                sst = spl.tile([128, TT], BF16, tag="sst",
                                               bufs=3, name="sst")
                                nc.scalar.copy(sst[:], tpp[:])
                                nc.sync.dma_start(
                                    gtm_d[tb * TT + ti * 128:
                                          tb * TT + (ti + 1) * 128, :],
                                    sst[:])

                        imp_pass()
                        spill_pass()

                # chain weights + gT freed here.
                with tc.tile_pool(name="mid", bufs=1) as mid:
                    selT = mid.tile([128, 8, NE], BF16)
                    posF = mid.tile([2, NE], F32)
                    posT = mid.tile([128, 4, 2], F32)
                    aT = mid.tile([128, 4], F32)
                    bT = mid.tile([128, 4], F32)
                    cT = mid.tile([128, 4], F32)
                    Xe = mid.tile([128, 4, 100], F32)
                    XP = mid.tile([128, 4, 100], F32)
                    Ye = mid.tile([128, 4, 100], F32)
                    Xb = mid.tile([128, 4, 100], BF16)
                    Yb = mid.tile([128, 4, 100], BF16)
                    YP8 = mid.tile([128, 4, 100], F8)
                    p0a = mid.tile([128, 4], F32)
                    p0s = mid.tile([128, 4], F32)
                    hT = mid.tile([128, 16, NE], BF16)
                    projf8 = mid.tile([128, 6, D], F8)
                    recM = mid.tile([100, 100], F32)
                    rtail = mid.tile([1, 240], F32)
                    dbuf = mid.tile([128, 6, GP], F8)
                    recrep = mid.tile([128, 20, TT], F32)
                    wo1 = mid.tile([128, 8, 2 * E], BF16)

                    # ---- selection (GPSIMD) + pass 2 ----
                    with tc.tile_pool(name="selg", bufs=1) as selg:
                        we2r = selg.tile([128, 4, EFR], BF16)
                        gsel = selg.tile([128, 4, NE], BF16)
                        sp = selg
                        # sync-queue priority: tiny pos slice of we2r first
                        # (unblocks the pass2 pos group + X/Y + e-fill).
                        # wo1 is issued AFTER a dummy idxr spill: its wait
                        # blocks SP.SEQ, deferring wo1's DMA-engine requests
                        # until the selection gather's request is in flight
                        # (arbitration is FIFO by request time).
                        nc.sync.dma_start(we2r[:, :, 1024:1026],
                                          we2_d[:, :, 1024:1026])
                        nc.sync.dma_start(we2r[:, :, 0:512],
                                          we2_d[:, :, 0:512])
                        nc.sync.dma_start(we2r[:, :, 512:1024],
                                          we2_d[:, :, 512:1024])
                        wps_ctx = tc.tile_pool(name="wps", bufs=2,
                                               space="PSUM")
                        wps = wps_ctx.__enter__()
                        # warm-keeper first: no selection deps, so PE stays
                        # busy (and clocked) while GPSIMD runs the top-k
                        for wi in range(14):
                            wt = wps.tile([1, 512], F32, tag="warm",
                                          name="wt")
                            nc.tensor.matmul(
                                wt[:], ident_bf[:, 0:1],
                                wo2[:, wi % 16, :], start=True, stop=True)
                        thr = sp.tile([1, 2], F32)
                        nc.gpsimd.kth_largest(thr[:], ipm[:], n_per_lane=32,
                                              k=510,
                                              quantile=1.0 - 510.5 / 4095.0)
                        iota1 = sp.tile([16, 256], F32)
                        nc.gpsimd.iota(iota1[:], pattern=[[1, 256]], base=1,
                                       channel_multiplier=256,
                                       allow_small_or_imprecise_dtypes=True)
                        thr_b = sp.tile([16, 1], F32)
                        nc.gpsimd.partition_broadcast(thr_b[:],
                                                      thr[0:1, 1:2])
                        # whole mask/compact chain stays on GPSIMD: no
                        # cross-engine semaphore hops on the critical path
                        mask = sp.tile([16, 256], F32)
                        nc.gpsimd.tensor_scalar(mask[:], iiv[:], thr_b[:],
                                                None, op0=ALU.is_ge)
                        selv = sp.tile([16, 256], F32)
                        nc.gpsimd.tensor_mul(selv[:], mask[:], iota1[:])
                        nc.gpsimd.tensor_scalar_add(selv[:], selv[:], -1.0)
                        sg = sp.tile([16, 32], F32)
                        nfound = sp.tile([1, 1], U32)
                        nc.gpsimd.sparse_gather(sg[:], selv[:],
                                                num_found=nfound[:])
                        idxr = sp.tile([128, 32], I16)
                        psx = wps.tile([128, 32], F32, tag="idx")
                        nc.tensor.matmul(psx[:], repmat[:], sg[:],
                                         start=True, stop=True)
                        nc.vector.tensor_copy(idxr[:], psx[:])

                        # bridge the gather wait at a warm clock
                        for wi in range(10):
                            wt = wps.tile([1, 512], F32, tag="warm",
                                          name="wt")
                            nc.tensor.matmul(
                                wt[:], ident_bf[:, 0:1],
                                wo2[:, wi % 16, :], start=True, stop=True)

                        nc.gpsimd.dma_gather(gsel[:], gtm_d[:], idxr[:],
                                             NE, NE, F, transpose=True)
                        wps_ctx.__exit__(None, None, None)
                        # idxr-dependent barrier, then wo1 (see above)
                        nc.sync.dma_start(scr_d[:], idxr[:])
                        for mc in range(8):
                            nc.sync.dma_start(
                                wo1[:, :, mc * 256:(mc + 1) * 256],
                                wo1_d[:, :, mc * 256:(mc + 1) * 256])

                        # ---- pass 2 ----
                        with tc.tile_pool(name="p2ps", bufs=3,
                                          space="PSUM") as p2ps, \
                             tc.tile_pool(name="pt_ps", bufs=1,
                                          space="PSUM") as pt_ps:
                            # positions group FIRST (unblocks X/Y/e-fill)
                            psP = pt_ps.tile([2, NE], F32, tag="pos")
                            for k in range(4):
                                nc.tensor.matmul(psP[:],
                                                 we2r[:, k, 1024:1026],
                                                 gsel[:, k, :],
                                                 start=(k == 0),
                                                 stop=(k == 3))
                            nc.vector.tensor_scalar(posF[:], psP[:],
                                                    be2[0:2, 8:9], None,
                                                    op0=ALU.add)
                            ptp = pt_ps.tile([128, 4, 2], F32, tag="ptp")
                            for n4 in range(4):
                                nc.tensor.transpose(
                                    ptp[:, n4, :],
                                    posF[0:2, n4 * 128:(n4 + 1) * 128],
                                    ident[0:2, 0:2])
                            nc.vector.tensor_copy(posT[:], ptp[:])
                            sqp = mid.tile([128, 4, 2], F32, name="sqp")
                            nc.scalar.activation(sqp[:], posT[:], AF.Square)
                            nc.vector.tensor_add(cT[:], sqp[:, :, 0],
                                                 sqp[:, :, 1])
                            nc.vector.tensor_scalar_mul(cT[:], cT[:], -10.0)
                            nc.vector.tensor_scalar_mul(aT[:], posT[:, :, 0],
                                                        20.0)
                            nc.vector.tensor_scalar_mul(bT[:], posT[:, :, 1],
                                                        20.0)
                            for n4 in range(4):
                                nc.scalar.activation(Xe[:, n4, :], gv[:],
                                                     AF.Exp,
                                                     scale=aT[:, n4:n4 + 1],
                                                     bias=cT[:, n4:n4 + 1])
                                nc.scalar.activation(Ye[:, n4, :], gv[:],
                                                     AF.Exp,
                                                     scale=bT[:, n4:n4 + 1])
                            nc.vector.tensor_copy(Xb[:], Xe[:])
                            nc.vector.tensor_copy(Yb[:], Ye[:])
                            nc.vector.tensor_scalar_add(XP[:], Xe[:], -1.0)
                            nc.scalar.activation(YP8[:], Ye[:], AF.Copy,
                                                 scale=DSC, bias=-DSC)
                            # pad zeroing (full partition range: engines
                            # need aligned bases; W2 rows overwrite later).
                            # Issued after the pass2-critical Act ops.
                            nc.scalar.memzero(dbuf[:, 0:4, G2:GP])
                            nc.scalar.memzero(projf8[:, 4:6, :])

                            # states groups
                            for fc in range(8):
                                ps = p2ps.tile([128, NE], F32, tag="mm2")
                                for k in range(4):
                                    nc.tensor.matmul(
                                        ps[:],
                                        we2r[:, k, fc * 128:(fc + 1) * 128],
                                        gsel[:, k, :], start=(k == 0),
                                        stop=(k == 3))
                                nc.scalar.activation(selT[:, fc, :], ps[:],
                                                     AF.Identity,
                                                     bias=be2[:, fc:fc + 1])

                            # den = X^T Y + corr -> rec/65536, roundtripped
                            # into a partition-replicated [128, 20, 512] tile
                            psD = pt_ps.tile([100, 100], F32, tag="den")
                            for k in range(4):
                                nc.tensor.matmul(psD[:], Xb[:, k, :],
                                                 Yb[:, k, :],
                                                 start=(k == 0),
                                                 stop=(k == 3))
                            nc.vector.tensor_add(recM[:], psD[:], corrM[:])
                            nc.vector.reciprocal(recM[:], recM[:])
                            nc.vector.tensor_scalar_mul(recM[:], recM[:],
                                                        1.0 / OSC)
                            nc.sync.dma_start(rec_d[0:G2], recM[:])
                            nc.vector.memset(rtail[:], 0.0)
                            nc.sync.dma_start(rec_d[G2:GP], rtail[:])
                            nc.sync.dma_start(
                                recrep[:],
                                rec_d.rearrange("(a b) -> a b", b=TT)
                                .unsqueeze(0).broadcast_to([128, 20, TT]))
                            # indicator k-tiles queue behind the rec writes
                            # on SP: the rec write's wait blocks SP.SEQ, so
                            # this big transfer can't jump the gather/wo1
                            nc.sync.dma_start(dbuf[:, 4:6, :], ind_d[:])

                    # ---- e-field fill: dbuf = fp8(16 * X' (*) Y) ----
                    # DVE: fused 3-D STT per (chunk, n4) straight to fp8.
                    # Pool can't run STT (walrus), so its chunks are a
                    # tensor_tensor mult into etmp + an Act fp8 quant.
                    def estt(c):
                        for n4 in range(4):
                            nc.vector.scalar_tensor_tensor(
                                dbuf[:, n4, c * 500:(c + 1) * 500]
                                .rearrange("p (a c) -> p a c", c=100),
                                XP[:, n4, c * 5:(c + 1) * 5].unsqueeze(-1)
                                .broadcast_to([128, 5, 100]),
                                DSC,
                                Ye[:, n4, :].unsqueeze(1)
                                .broadcast_to([128, 5, 100]),
                                op0=ALU.mult, op1=ALU.mult)

                    with tc.tile_pool(name="etp", bufs=1) as etp:
                        etmps = {}

                        def emult(c):
                            etmp = etp.tile([128, 4, 500], F32, tag="etg",
                                            bufs=2, name=f"etmp{c}")
                            etmps[c] = etmp
                            for n4 in range(4):
                                nc.gpsimd.tensor_mul(
                                    etmp[:, n4, :]
                                    .rearrange("p (a c) -> p a c", c=100),
                                    XP[:, n4, c * 5:(c + 1) * 5]
                                    .unsqueeze(-1)
                                    .broadcast_to([128, 5, 100]),
                                    Ye[:, n4, :].unsqueeze(1)
                                    .broadcast_to([128, 5, 100]))

                        def equant(c):
                            nc.scalar.activation(
                                dbuf[:, 0:4, c * 500:(c + 1) * 500],
                                etmps.pop(c)[:], AF.Copy, scale=DSC)

                        for c in range(13):
                            estt(c)
                        emult(13)
                        emult(14)

                        # ---- wo1 ----
                        with tc.tile_pool(name="ops1", bufs=4,
                                          space="PSUM") as ops1:
                            for m in range(16):
                                ps = ops1.tile([128, NE], F32, tag="mmh")
                                for c in range(8):
                                    nc.tensor.matmul(
                                        ps[:],
                                        wo1[:, c, m * 128:(m + 1) * 128],
                                        selT[:, c, :], start=(c == 0),
                                        stop=(c == 7))
                                nc.scalar.activation(hT[:, m, :], ps[:],
                                                     AF.Gelu,
                                                     bias=bo1[:, m:m + 1])
                                if m % 2 == 1 and m <= 13:
                                    c0 = 13 + (m - 1) // 2
                                    equant(c0)
                                    if c0 + 2 < 20:
                                        emult(c0 + 2)

                    # ---- wo2 + attn, interleaved per dim-chunk m ----
                    with tc.tile_pool(name="oops", bufs=1,
                                      space="PSUM") as oops, \
                         tc.tile_pool(name="oo", bufs=4) as oo, \
                         tc.tile_pool(name="pjm", bufs=2) as pjm:
                        ops2 = opst = w2ps = oops
                        for m in range(4):
                            msl = slice(m * 128, (m + 1) * 128)
                            psW = ops2.tile([128, NE], F32, tag="mmp",
                                            bufs=1, name="psW")
                            for k in range(16):
                                nc.tensor.matmul(
                                    psW[:], wo2[:, k, msl],
                                    hT[:, k, :], start=(k == 0),
                                    stop=(k == 15))
                            projTm = pjm.tile([128, NE], BF16, tag="pjT")
                            nc.vector.tensor_scalar(
                                projTm[:], psW[:], bo2[:, m:m + 1], 0.0,
                                op0=ALU.add, op1=ALU.add,
                                accum_out=p0a[:, m:m + 1])
                            nc.vector.tensor_scalar_mul(
                                p0s[:, m:m + 1], p0a[:, m:m + 1], OSC)
                            for n4 in range(4):
                                pstp = opst.tile([128, 128], BF16,
                                                 tag="ptb", bufs=1,
                                                 name="pstp")
                                nc.tensor.transpose(
                                    pstp[:],
                                    projTm[:, n4 * 128:(n4 + 1) * 128],
                                    ident_bf[:])
                                nc.scalar.activation(
                                    projf8[:, n4, msl], pstp[:],
                                    AF.Copy, scale=PSC)
                            # W2 = Y'^T proj for this dim-chunk, hi/lo fp8
                            psW2 = w2ps.tile([100, 128], F32, tag="w2",
                                             bufs=1, name="psW2")
                            for k in range(4):
                                nc.tensor.matmul(
                                    psW2[:], YP8[:, k, :],
                                    projf8[:, k, msl],
                                    start=(k == 0), stop=(k == 3))
                            nc.scalar.activation(
                                projf8[0:100, 4, msl], psW2[:],
                                AF.Copy, scale=1.0 / 32.0)
                            tW2 = pjm.tile([100, 128], F32, tag="tw2")
                            nc.vector.scalar_tensor_tensor(
                                tW2[:], projf8[0:100, 4, msl], -32.0,
                                psW2[:], op0=ALU.mult, op1=ALU.add)
                            nc.vector.tensor_scalar(
                                projf8[0:100, 5, msl], tW2[:], 0.125, None,
                                op0=ALU.mult)

                            ngq, nper = 10, 2
                            for gq in range(ngq):
                                ot = oo.tile([128, nper, TT], BF16,
                                             tag="ot", name="ot")
                                for g2 in range(nper):
                                    gt = gq * nper + g2
                                    pso = oops.tile([128, TT], F32, tag="mmo",
                                                    bufs=5, name="pso")
                                    for c in range(3):
                                        nc.tensor.matmul(
                                            pso[:],
                                            projf8[:, 2 * c:2 * c + 2, msl],
                                            dbuf[:, 2 * c:2 * c + 2,
                                                 gt * TT:(gt + 1) * TT],
                                            start=(c == 0), stop=(c == 2),
                                            perf_mode=DR)
                                    # DVE fused STT takes 12 of 20; Pool
                                    # can't read PSUM, so its 8 tiles get an
                                    # Act Identity(psum+P0) drain first
                                    if gt % 5 < 3:
                                        nc.vector.scalar_tensor_tensor(
                                            ot[:, g2, :], pso[:],
                                            p0s[:, m:m + 1],
                                            recrep[:, gt, :],
                                            op0=ALU.add, op1=ALU.mult)
                                    else:
                                        ptm = oo.tile([128, TT], F32,
                                                      tag="ptm", bufs=2,
                                                      name="ptm")
                                        nc.scalar.activation(
                                            ptm[:], pso[:], AF.Identity,
                                            bias=p0s[:, m:m + 1])
                                        nc.gpsimd.tensor_mul(
                                            ot[:, g2, :], ptm[:],
                                            recrep[:, gt, :])
                                dq = nc.sync if gq % 2 == 0 else nc.scalar
                                dq.dma_start(
                                    out_d[msl, gq * nper * TT:
                                          (gq + 1) * nper * TT], ot[:])
    nc.compile()
    return nc


_NC_CACHE = {}


def _host_inputs(inputs):
    """Replicated host-side tensor prep (layout shuffles only)."""
    f32 = np.float32
    bf = ml_dtypes.bfloat16
    f8 = ml_dtypes.float8_e4m3fn
    W1 = np.asarray(inputs["W1"], f32)
    W2 = np.asarray(inputs["W2"], f32)
    Wc = np.asarray(inputs["Wc"], f32)
    We1 = np.asarray(inputs["We1"], f32)
    We2 = np.asarray(inputs["We2"], f32)
    Wo1 = np.asarray(inputs["Wo1"], f32)
    Wo2 = np.asarray(inputs["Wo2"], f32)
    b1 = np.asarray(inputs["b1"], f32); b2 = np.asarray(inputs["b2"], f32)
    bc = np.asarray(inputs["bc"], f32); be1 = np.asarray(inputs["be1"], f32)
    be2 = np.asarray(inputs["be2"], f32)
    bo1 = np.asarray(inputs["bo1"], f32); bo2 = np.asarray(inputs["bo2"], f32)

    def kchunk(w, nk):   # [K, N] -> [128, nk, N]
        return np.ascontiguousarray(
            w.reshape(nk, 128, w.shape[1]).transpose(1, 0, 2))

    def bvec(b, ncol):   # [N] -> [128, ncol]
        return np.ascontiguousarray(b.reshape(ncol, 128).T)

    # fold coords GEMM: fi@We1 = df@(We1[2:] + Wc@We1[:2]) + (be1 + bc@We1[:2])
    We1_64 = We1.astype(np.float64)
    we1_eff = (We1_64[2:] + Wc.astype(np.float64) @ We1_64[:2]).astype(f32)
    be1_eff = (be1.astype(np.float64)
               + bc.astype(np.float64) @ We1_64[:2]).astype(f32)

    We2_64 = We2.astype(np.float64)
    Aq = (We2_64 @ We2_64.T).astype(f32)               # [512, 512]
    vq = (2.0 * (We2_64 @ be2.astype(np.float64))).astype(f32)  # [512]

    # reorder We2 columns: [states (cols 2..1025) | px, py]; drop col 1026
    perm = np.concatenate([np.arange(2, 2 + E), np.array([0, 1])])
    we2r = np.ascontiguousarray(We2[:, perm])          # [512, 1026]
    be2r = np.zeros((9, 128), f32)
    be2r.ravel()[:E] = be2[2:2 + E]
    be2r[8, 0:2] = be2[0:2]
    be2r = np.ascontiguousarray(be2r.T)                # [128, 9]

    g = np.linspace(-1.0, 1.0, 100, dtype=np.float64)
    gv = np.ascontiguousarray(
        np.broadcast_to(g.astype(f32), (128, 100)))
    corrM = (1e-8 * np.exp(10.0 * (g[:, None] ** 2 + g[None, :] ** 2))
             ).astype(f32)

    # iy-indicator k-tiles for the W2 hi/lo rows of the attn GEMM
    iy = np.arange(GP) % 100
    ind = np.zeros((128, 2, GP), f8)
    eye = (np.arange(128)[:, None] == iy[None, :])
    ind[:, 0, :] = np.where(eye, np.float32(32.0), 0).astype(f8)
    ind[:, 1, :] = np.where(eye, np.float32(8.0), 0).astype(f8)

    return {
        "aq": kchunk(Aq, 4), "vq": bvec(vq, 4),
        "w1": kchunk(W1, 4), "w2": kchunk(W2, 4),
        "we1": kchunk(we1_eff, 4),
        "we2r": kchunk(we2r, 4).astype(bf),
        "wo1": kchunk(Wo1, 8).astype(bf),
        "wo2": kchunk(Wo2, 16).astype(bf),
        "b1": bvec(b1, 4), "b2": bvec(b2, 4),
        "be1": bvec(be1_eff, 4), "be2r": be2r,
        "bo1": bvec(bo1, 16), "bo2": bvec(bo2, 4),
        "gv": gv, "corrM": corrM, "ind": ind,
    }


def kernel(**inputs):
    if CHAIN not in _NC_CACHE:
        _NC_CACHE[CHAIN] = _build(CHAIN)
    nc = _NC_CACHE[CHAIN]
    shared = _host_inputs(inputs)
    x = np.asarray(inputs["x"], np.float32)
    in_maps = []
    for b in range(B):
        m = dict(shared)
        m["xT"] = np.ascontiguousarray(x[b].T)
        in_maps.append(m)
    res = run_bass_kernel_spmd(nc, in_maps, core_ids=list(range(B)))
    return np.stack([np.asarray(r["out"]).astype(np.float32)[:, :G2].T
                     for r in res.results])


# revision 81
# speedup vs baseline: 1.0185x; 1.0185x over previous
"""Trainium2 Bass kernel for nn_DFNPureModel (retrieval_knn).

Data-parallel over batch B=8 across 8 NeuronCores; params replicated.
Per core (one batch element, S=4096 tokens, D=512, F=512, E=1024, Ne=512,
G2=10000 grid points):

  chain (f32r, feature-major):  h1 = gelu(x@W1), df = gelu(h1@W2),
           g = gelu(df@We1eff)  with  We1eff = We1[2:] + Wc@We1[:2].
           importance^2 = g^T A g + v^T g with A = We2 We2^T (host).
           In-chain: g is copied to bf16 (Act), PE-transposed to token-major
           and spilled to DRAM so selection gathers with no post-chain pass.
  top-k:   kth_largest (GPSIMD) -> mask+iota+sparse_gather -> token idx;
           dma_gather (transpose) -> g_sel feature-major bf16.
  pass 2:  ef_sel = g_sel@We2r: We2 columns reordered on host so the states
           land 128-aligned (wo1 contracts exactly 8 chunks) and the
           positions form a tiny separate group computed FIRST.
  attention (the big rewrite): positions cluster near the origin, so with
           X = exp(20 px g + c), Y = exp(20 py g), X' = X-1, Y' = Y-1:
             e[n,g] = X[n,ix]Y[n,iy],   e = 1 + X'Y + Y'
             out*den = P0 + (X'Y)^T proj + W2[iy(g),:],  W2 = Y'^T proj
           P0 = colsum(proj) rides the wo2 bias-add (accum_out, exact f32).
           The (X'Y)-field quantizes to fp8e4m3 at x16 (errors scale with
           |X'Y| ~ 0.1, so fp8 beats the old bf16-e by ~5x) and the big GEMM
           runs MatmulPerfMode.DoubleRow (2 k-tiles per matmul).  The W2
           term enters the SAME fp8 GEMM through two constant indicator
           k-tiles (values 32/8) against a hi/lo fp8 split of W2 - one
           extra DoubleRow matmul per tile, no elementwise work.
           The e-field fill is fp8(16*X' (*) Y) per 500-grid chunk:
           fused 3-D-broadcast STTs on DVE (13 chunks) and tensor_mul +
           Act-quant pairs on Pool (7 chunks), no PE.
           den = X^T Y + corr: one rank-100 GEMM; rec = 1/den replicated
           across partitions by a stride-0 DMA roundtrip; the combine is
           (psum + 65536*P0) * rec/65536 -> bf16: a fused STT on DVE for
           12/20 tiles, an Act Identity psum-drain + Pool tensor_mul for
           the rest (Pool cannot read PSUM).
           Output is written transposed [D, G] in bf16 (host does .T).
  overlap: e-fill runs under wo1; wo2 and attn interleave per dim-chunk so
           out DMA (alternating queues) starts early; PE prewarm covers the
           initial x/w1 DMA; a warm-keeper bridges the selection gap, and a
           dummy idxr-dependent DMA defers wo1's transfers so the gather
           wins the FIFO DMA-engine arbitration.

Known TRN2 hazards handled: f32r matmul needs rounded producers and crashes
for 1<M<128 (only M=1/M=128 used); memset cannot write f32r; partition-dim
stride-0 APs are rejected by engines (the rec replicate rides a DMA, which
allows them on the DRAM side); walrus rejects STT/psum-reads on Pool, 4-D
STT access patterns, engine partition bases off 0/32/64/96, and accum_out
without an explicit op1; DMA waits hold the issuing sequencer, so dependent
DMAs alternate queues.
"""

import numpy as np
import ml_dtypes

import concourse.bass as bass
import concourse.mybir as mybir
import concourse.tile as tile
from concourse import bacc
from concourse.bass_utils import run_bass_kernel_spmd
from concourse.masks import make_identity

F32 = mybir.dt.float32
F32R = mybir.dt.float32r
BF16 = mybir.dt.bfloat16
F8 = mybir.dt.float8e4
I16 = mybir.dt.int16
U32 = mybir.dt.uint32
AF = mybir.ActivationFunctionType
ALU = mybir.AluOpType
DR = mybir.MatmulPerfMode.DoubleRow

B, S, D, F, E, NE = 8, 4096, 512, 512, 1024, 512
EFR = 1026          # reordered We2 cols: 1024 states + 2 pos
G2 = 10000
GP = 10240          # padded grid (20 chunks of 512)
TB = 8              # token blocks
TT = 512            # tokens per block

DSC = 16.0          # fp8 scale on the X'Y field
PSC = 4096.0        # fp8 scale on proj
OSC = DSC * PSC     # psum scale
CHAIN = "f32r"


def _build(chain=CHAIN):
    nc = bacc.Bacc("TRN2", target_bir_lowering=False, debug=False,
                   enable_asserts=True, num_devices=8)

    def din(name, shape, dt):
        return nc.dram_tensor(name, list(shape), dt, kind="ExternalInput").ap()

    x_d = din("xT", [D, S], F32R)       # host-transposed x
    a_d = din("aq", [128, 4, F], F32R)  # A = We2 We2^T (importance quadratic)
    v_d = din("vq", [128, 4], F32)      # v = 2 We2 be2
    w1_d = din("w1", [128, 4, F], F32R)
    w2_d = din("w2", [128, 4, F], F32R)
    we1_d = din("we1", [128, 4, F], F32R)   # We1[2:] + Wc@We1[:2]
    we2_d = din("we2r", [128, 4, EFR], BF16)  # reordered: states | pos
    wo1_d = din("wo1", [128, 8, 2 * E], BF16)  # unpadded, states-aligned
    wo2_d = din("wo2", [128, 16, D], BF16)
    b1_d = din("b1", [128, 4], F32)
    b2_d = din("b2", [128, 4], F32)
    be1_d = din("be1", [128, 4], F32)   # be1 + bc @ We1[:2]
    be2_d = din("be2r", [128, 9], F32)  # chunks 0-7 states bias; col 8 pos
    bo1_d = din("bo1", [128, 16], F32)
    bo2_d = din("bo2", [128, 4], F32)
    gv_d = din("gv", [128, 100], F32)   # grid values replicated
    corr_d = din("corrM", [100, 100], F32)  # 1e-8*exp(10(gx^2+gy^2))
    ind_d = din("ind", [128, 2, GP], F8)    # 32/8 iy-indicator k-tiles
    out_d = nc.dram_tensor("out", [D, GP], BF16, kind="ExternalOutput").ap()
    # token-major bf16 spill of g for the selection dma_gather
    gtm_d = nc.dram_tensor("gtm", [S, F], BF16, kind="Internal").ap()
    rec_d = nc.dram_tensor("recd", [GP], F32, kind="Internal").ap()
    scr_d = nc.dram_tensor("scrd", [128, 32], I16, kind="Internal").ap()

    with tile.TileContext(nc) as tc:
        with tc.tile_pool(name="small", bufs=1) as small:
            b1 = small.tile([128, 4], F32)
            b2 = small.tile([128, 4], F32)
            be1 = small.tile([128, 4], F32)
            be2 = small.tile([128, 9], F32)
            bo1 = small.tile([128, 16], F32)
            bo2 = small.tile([128, 4], F32)
            gv = small.tile([128, 100], F32)
            corrM = small.tile([100, 100], F32)
            ident = small.tile([128, 128], F32)
            make_identity(nc, ident[:])
            ones_f32 = small.tile([128, 1], F32)
            nc.vector.memset(ones_f32[:], 1.0)
            ones_col = small.tile([128, 1], F32R)
            nc.vector.tensor_copy(ones_col[:], ones_f32[:])
            vq = small.tile([128, 4], F32)
            # importance^2 in both selection layouts, filled per token block
            ipm = small.tile([128, 32], F32)
            iiv = small.tile([16, 256], F32)
            ident_bf = small.tile([128, 128], BF16)
            nc.vector.tensor_copy(ident_bf[:], ident[:])
            repmat = small.tile([16, 128], F32)
            for c in range(8):
                nc.vector.tensor_copy(repmat[:, c * 16:(c + 1) * 16],
                                      ident[0:16, 0:16])

            with tc.tile_pool(name="ob2", bufs=1) as ob2:
                # output-side weights, transferred during the chain (also
                # feeds the selection-window warm-keeper matmuls)
                wo2 = ob2.tile([128, 16, D], BF16)

                # ============ chain ============
                with tc.tile_pool(name="gbuf", bufs=1) as gbuf, \
                     tc.tile_pool(name="wts", bufs=1) as wts, \
                     tc.tile_pool(name="pa", bufs=2) as pa, \
                     tc.tile_pool(name="pa1", bufs=1) as pa1, \
                     tc.tile_pool(name="spl", bufs=1) as spl, \
                     tc.tile_pool(name="mm_ps", bufs=3, space="PSUM") as mm_ps, \
                     tc.tile_pool(name="imp_ps", bufs=1, space="PSUM") as imp_ps, \
                     tc.tile_pool(name="tp_ps", bufs=2, space="PSUM") as tp_ps:
                    gT = gbuf.tile([128, 4, S], F32R)
                    w1 = wts.tile([128, 4, F], F32R)
                    w2 = wts.tile([128, 4, F], F32R)
                    we1 = wts.tile([128, 4, F], F32R)
                    aq = wts.tile([128, 4, F], F32R)

                    xts = {}

                    def load_x(tb):
                        t = pa.tile([128, 4, TT], F32R, tag="xT")
                        nc.sync.dma_start(
                            t[:], x_d.rearrange("(c p) t -> p c t", p=128)
                            [:, :, tb * TT:(tb + 1) * TT])
                        xts[tb] = t

                    # every transfer the chain depends on rides ONE queue in
                    # priority order (DMA engines serve a single transfer at
                    # a time, in acquisition order).
                    xt0 = pa.tile([128, 4, TT], F32R, tag="xT")
                    xr = x_d.rearrange("(c p) t -> p c t", p=128)
                    for kh in range(2):
                        ks = slice(2 * kh, 2 * kh + 2)
                        nc.sync.dma_start(w1[:, ks, :], w1_d[:, ks, :])
                        nc.sync.dma_start(xt0[:, ks, :], xr[:, ks, 0:TT])
                    xts[0] = xt0
                    nc.sync.dma_start(b1[:], b1_d[:])
                    nc.sync.dma_start(b2[:], b2_d[:])
                    for kh in range(2):
                        ks = slice(2 * kh, 2 * kh + 2)
                        nc.sync.dma_start(w2[:, ks, :], w2_d[:, ks, :])
                    nc.sync.dma_start(be1[:], be1_d[:])
                    for kh in range(2):
                        ks = slice(2 * kh, 2 * kh + 2)
                        nc.sync.dma_start(we1[:, ks, :], we1_d[:, ks, :])
                    for kh in range(2):
                        ks = slice(2 * kh, 2 * kh + 2)
                        nc.sync.dma_start(aq[:, ks, :], a_d[:, ks, :])
                    load_x(1)
                    nc.sync.dma_start(vq[:], v_d[:])
                    nc.sync.dma_start(be2[:], be2_d[:])
                    nc.scalar.dma_start(wo2[:, 0:8, :], wo2_d[:, 0:8, :])
                    nc.scalar.dma_start(wo2[:, 8:16, :], wo2_d[:, 8:16, :])
                    nc.scalar.dma_start(gv[:], gv_d[:])
                    nc.scalar.dma_start(corrM[:], corr_d[:])
                    nc.scalar.dma_start(bo1[:], bo1_d[:])
                    nc.scalar.dma_start(bo2[:], bo2_d[:])

                    # prewarm: keep PE busy from ~t=0.3us so the chain starts
                    # at a ramped clock instead of pstate-low
                    for wi in range(3):
                        wt = mm_ps.tile([128, 128], F32, tag="mm", name="wt")
                        nc.tensor.matmul(wt[:], ident[:], ident[:],
                                         start=True, stop=True)

                    for tb in range(TB):
                        tok = slice(tb * TT, (tb + 1) * TT)
                        xT = xts.pop(tb)
                        if tb + 1 < TB and tb > 0:
                            load_x(tb + 1)

                        h1g = pa.tile([128, 4, TT], F32R, tag="h1g")
                        if tb == 0:
                            # k-outer in two m-halves: matmuls start when the
                            # first w1/x0 chunk lands
                            for half in range(2):
                                pss = [mm_ps.tile([128, TT], F32,
                                                  tag=f"mm0{m}", bufs=1,
                                                  name=f"ps0{m}")
                                       for m in range(2)]
                                for k in range(4):
                                    for m2 in range(2):
                                        m = half * 2 + m2
                                        nc.tensor.matmul(
                                            pss[m2][:],
                                            w1[:, k, m * 128:(m + 1) * 128],
                                            xT[:, k, :], start=(k == 0),
                                            stop=(k == 3))
                                for m2 in range(2):
                                    m = half * 2 + m2
                                    nc.scalar.activation(
                                        h1g[:, m, :], pss[m2][:], AF.Gelu,
                                        bias=b1[:, m:m + 1])
                        else:
                            for m in range(4):
                                ps = mm_ps.tile([128, TT], F32, tag="mm")
                                for k in range(4):
                                    nc.tensor.matmul(
                                        ps[:], w1[:, k, m * 128:(m + 1) * 128],
                                        xT[:, k, :], start=(k == 0),
                                        stop=(k == 3))
                                nc.scalar.activation(h1g[:, m, :], ps[:],
                                                     AF.Gelu,
                                                     bias=b1[:, m:m + 1])

                        dfg = pa.tile([128, 4, TT], F32R, tag="dfg")
                        for m in range(4):
                            ps = mm_ps.tile([128, TT], F32, tag="mm")
                            for k in range(4):
                                nc.tensor.matmul(
                                    ps[:], w2[:, k, m * 128:(m + 1) * 128],
                                    h1g[:, k, :], start=(k == 0), stop=(k == 3))
                            nc.scalar.activation(dfg[:, m, :], ps[:], AF.Gelu,
                                                 bias=b2[:, m:m + 1])

                        gbf = spl.tile([128, 4, TT], BF16, tag="gbf", bufs=2)
                        for m in range(4):
                            ps = mm_ps.tile([128, TT], F32, tag="mm")
                            for k in range(4):
                                nc.tensor.matmul(
                                    ps[:], we1[:, k, m * 128:(m + 1) * 128],
                                    dfg[:, k, :], start=(k == 0), stop=(k == 3))
                            nc.scalar.activation(gT[:, m, tok], ps[:], AF.Gelu,
                                                 bias=be1[:, m:m + 1])
                            nc.scalar.activation(gbf[:, m, :],
                                                 gT[:, m, tok].bitcast(F32),
                                                 AF.Copy)

                        def imp_pass():
                            # importance^2 = g^T A g + v^T g; the elementwise
                            # (Ag+v)*g splits DVE/Pool so the A-GEMM's psum
                            # rotation never throttles on one engine; psi
                            # accumulates the two halves
                            psi = imp_ps.tile([1, TT], F32, tag="psi",
                                              name="psi")
                            accs = []
                            for h in range(2):
                                acc = pa.tile([128, TT], F32R,
                                              tag=f"prod{h}", name="acc")
                                accs.append(acc)
                                for m2 in range(2):
                                    m = 2 * h + m2
                                    ps = mm_ps.tile([128, TT], F32, tag="mm",
                                                    name="ps")
                                    for k in range(4):
                                        nc.tensor.matmul(
                                            ps[:],
                                            aq[:, k, m * 128:(m + 1) * 128],
                                            gT[:, k, tok],
                                            start=(k == 0), stop=(k == 3))
                                    # Pool can't read PSUM (walrus), so both
                                    # halves run the fused STT on DVE
                                    dst = acc
                                    if m2 == 1:
                                        dst = pa.tile([128, TT], F32R,
                                                      tag=f"prodm{h}",
                                                      name="prod")
                                    nc.vector.scalar_tensor_tensor(
                                        dst[:], ps[:], vq[:, m:m + 1],
                                        gT[:, m, tok].bitcast(F32),
                                        op0=ALU.add, op1=ALU.mult)
                                    if m2 == 1:
                                        nc.vector.tensor_add(
                                            acc[:], acc[:].bitcast(F32),
                                            dst[:].bitcast(F32))
                            for h in range(2):
                                nc.tensor.matmul(psi[:], ones_col[:],
                                                 accs[h][:], start=(h == 0),
                                                 stop=(h == 1))
                            imp_tb = pa1.tile([1, TT], F32, tag="imp_tb",
                                              bufs=2, name="imp_tb")
                            nc.scalar.copy(imp_tb[:], psi[:])
                            nc.sync.dma_start(
                                ipm[16 * tb:16 * (tb + 1), :],
                                imp_tb[0:1, :].rearrange("p (a b) -> p a b",
                                                         a=16))
                            nc.sync.dma_start(iiv[2 * tb:2 * (tb + 1), :],
                                              imp_tb[0:1, :])

                        def spill_pass():
                            # in-chain token-major bf16 spill (PE transposes)
                            for ti in range(4):
                                tpp = tp_ps.tile([128, TT], BF16, tag="tp",
                                                 name="tpp")
                                for k in range(4):
                                    nc.tensor.transpose(
                                        tpp[:, k * 128:(k + 1) * 128],
                                        gbf[:, k, ti * 128:(ti + 1) * 128],
                                        ident_bf[:])
                                sst = spl.tile([128, TT], BF16, tag="sst",
                                               bufs=3, name="sst")
                                nc.scalar.copy(sst[:], tpp[:])
                                nc.sync.dma_start(
                                    gtm_d[tb * TT + ti * 128:
                                          tb * TT + (ti + 1) * 128, :],
                                    sst[:])

                        imp_pass()
                        spill_pass()

                # chain weights + gT freed here.
                with tc.tile_pool(name="mid", bufs=1) as mid:
                    selT = mid.tile([128, 8, NE], BF16)
                    posF = mid.tile([2, NE], F32)
                    posT = mid.tile([128, 4, 2], F32)
                    aT = mid.tile([128, 4], F32)
                    bT = mid.tile([128, 4], F32)
                    cT = mid.tile([128, 4], F32)
                    Xe = mid.tile([128, 4, 100], F32)
                    XP = mid.tile([128, 4, 100], F32)
                    Ye = mid.tile([128, 4, 100], F32)
                    Xb = mid.tile([128, 4, 100], BF16)
                    Yb = mid.tile([128, 4, 100], BF16)
                    YP8 = mid.tile([128, 4, 100], F8)
                    p0a = mid.tile([128, 4], F32)
                    p0s = mid.tile([128, 4], F32)
                    hT = mid.tile([128, 16, NE], BF16)
                    projf8 = mid.tile([128, 6, D], F8)
                    recM = mid.tile([100, 100], F32)
                    rtail = mid.tile([1, 240], F32)
                    dbuf = mid.tile([128, 6, GP], F8)
                    recrep = mid.tile([128, 20, TT], F32)
                    wo1 = mid.tile([128, 8, 2 * E], BF16)

                    # ---- selection (GPSIMD) + pass 2 ----
                    with tc.tile_pool(name="selg", bufs=1) as selg:
                        we2r = selg.tile([128, 4, EFR], BF16)
                        gsel = selg.tile([128, 4, NE], BF16)
                        sp = selg
                        # sync-queue priority: tiny pos slice of we2r first
                        # (unblocks the pass2 pos group + X/Y + e-fill).
                        # wo1 is issued AFTER a dummy idxr spill: its wait
                        # blocks SP.SEQ, deferring wo1's DMA-engine requests
                        # until the selection gather's request is in flight
                        # (arbitration is FIFO by request time).
                        nc.sync.dma_start(we2r[:, :, 1024:1026],
                                          we2_d[:, :, 1024:1026])
                        nc.sync.dma_start(we2r[:, :, 0:512],
                                          we2_d[:, :, 0:512])
                        nc.sync.dma_start(we2r[:, :, 512:1024],
                                          we2_d[:, :, 512:1024])
                        wps_ctx = tc.tile_pool(name="wps", bufs=2,
                                               space="PSUM")
                        wps = wps_ctx.__enter__()
                        # warm-keeper first: no selection deps, so PE stays
                        # busy (and clocked) while GPSIMD runs the top-k
                        for wi in range(14):
                            wt = wps.tile([1, 512], F32, tag="warm",
                                          name="wt")
                            nc.tensor.matmul(
                                wt[:], ident_bf[:, 0:1],
                                wo2[:, wi % 16, :], start=True, stop=True)
                        thr = sp.tile([1, 2], F32)
                        nc.gpsimd.kth_largest(thr[:], ipm[:], n_per_lane=32,
                                              k=510,
                                              quantile=1.0 - 510.5 / 4095.0)
                        iota1 = sp.tile([16, 256], F32)
                        nc.gpsimd.iota(iota1[:], pattern=[[1, 256]], base=1,
                                       channel_multiplier=256,
                                       allow_small_or_imprecise_dtypes=True)
                        thr_b = sp.tile([16, 1], F32)
                        nc.gpsimd.partition_broadcast(thr_b[:],
                                                      thr[0:1, 1:2])
                        # whole mask/compact chain stays on GPSIMD: no
                        # cross-engine semaphore hops on the critical path
                        mask = sp.tile([16, 256], F32)
                        nc.gpsimd.tensor_scalar(mask[:], iiv[:], thr_b[:],
                                                None, op0=ALU.is_ge)
                        selv = sp.tile([16, 256], F32)
                        nc.gpsimd.tensor_mul(selv[:], mask[:], iota1[:])
                        nc.gpsimd.tensor_scalar_add(selv[:], selv[:], -1.0)
                        sg = sp.tile([16, 32], F32)
                        nfound = sp.tile([1, 1], U32)
                        nc.gpsimd.sparse_gather(sg[:], selv[:],
                                                num_found=nfound[:])
                        idxr = sp.tile([128, 32], I16)
                        psx = wps.tile([128, 32], F32, tag="idx")
                        nc.tensor.matmul(psx[:], repmat[:], sg[:],
                                         start=True, stop=True)
                        nc.vector.tensor_copy(idxr[:], psx[:])

                        # bridge the gather wait at a warm clock
                        for wi in range(10):
                            wt = wps.tile([1, 512], F32, tag="warm",
                                          name="wt")
                            nc.tensor.matmul(
                                wt[:], ident_bf[:, 0:1],
                                wo2[:, wi % 16, :], start=True, stop=True)

                        nc.gpsimd.dma_gather(gsel[:], gtm_d[:], idxr[:],
                                             NE, NE, F, transpose=True)
                        wps_ctx.__exit__(None, None, None)
                        # idxr-dependent barrier, then wo1 (see above)
                        nc.sync.dma_start(scr_d[:], idxr[:])
                        for mc in range(8):
                            nc.sync.dma_start(
                                wo1[:, :, mc * 256:(mc + 1) * 256],
                                wo1_d[:, :, mc * 256:(mc + 1) * 256])

                        # ---- pass 2 ----
                        with tc.tile_pool(name="p2ps", bufs=3,
                                          space="PSUM") as p2ps, \
                             tc.tile_pool(name="pt_ps", bufs=1,
                                          space="PSUM") as pt_ps:
                            # positions group FIRST (unblocks X/Y/e-fill)
                            psP = pt_ps.tile([2, NE], F32, tag="pos")
                            for k in range(4):
                                nc.tensor.matmul(psP[:],
                                                 we2r[:, k, 1024:1026],
                                                 gsel[:, k, :],
                                                 start=(k == 0),
                                                 stop=(k == 3))
                            nc.vector.tensor_scalar(posF[:], psP[:],
                                                    be2[0:2, 8:9], None,
                                                    op0=ALU.add)
                            ptp = pt_ps.tile([128, 4, 2], F32, tag="ptp")
                            for n4 in range(4):
                                nc.tensor.transpose(
                                    ptp[:, n4, :],
                                    posF[0:2, n4 * 128:(n4 + 1) * 128],
                                    ident[0:2, 0:2])
                            nc.vector.tensor_copy(posT[:], ptp[:])
                            sqp = mid.tile([128, 4, 2], F32, name="sqp")
                            nc.scalar.activation(sqp[:], posT[:], AF.Square)
                            nc.vector.tensor_add(cT[:], sqp[:, :, 0],
                                                 sqp[:, :, 1])
                            nc.vector.tensor_scalar_mul(cT[:], cT[:], -10.0)
                            nc.vector.tensor_scalar_mul(aT[:], posT[:, :, 0],
                                                        20.0)
                            nc.vector.tensor_scalar_mul(bT[:], posT[:, :, 1],
                                                        20.0)
                            for n4 in range(4):
                                nc.scalar.activation(Xe[:, n4, :], gv[:],
                                                     AF.Exp,
                                                     scale=aT[:, n4:n4 + 1],
                                                     bias=cT[:, n4:n4 + 1])
                                nc.scalar.activation(Ye[:, n4, :], gv[:],
                                                     AF.Exp,
                                                     scale=bT[:, n4:n4 + 1])
                            nc.vector.tensor_copy(Xb[:], Xe[:])
                            nc.vector.tensor_copy(Yb[:], Ye[:])
                            nc.vector.tensor_scalar_add(XP[:], Xe[:], -1.0)
                            nc.scalar.activation(YP8[:], Ye[:], AF.Copy,
                                                 scale=DSC, bias=-DSC)
                            # pad zeroing (full partition range: engines
                            # need aligned bases; W2 rows overwrite later).
                            # Issued after the pass2-critical Act ops.
                            nc.scalar.memzero(dbuf[:, 0:4, G2:GP])
                            nc.scalar.memzero(projf8[:, 4:6, :])

                            # states groups
                            for fc in range(8):
                                ps = p2ps.tile([128, NE], F32, tag="mm2")
                                for k in range(4):
                                    nc.tensor.matmul(
                                        ps[:],
                                        we2r[:, k, fc * 128:(fc + 1) * 128],
                                        gsel[:, k, :], start=(k == 0),
                                        stop=(k == 3))
                                nc.scalar.activation(selT[:, fc, :], ps[:],
                                                     AF.Identity,
                                                     bias=be2[:, fc:fc + 1])

                            # den = X^T Y + corr -> rec/65536, roundtripped
                            # into a partition-replicated [128, 20, 512] tile
                            psD = pt_ps.tile([100, 100], F32, tag="den")
                            for k in range(4):
                                nc.tensor.matmul(psD[:], Xb[:, k, :],
                                                 Yb[:, k, :],
                                                 start=(k == 0),
                                                 stop=(k == 3))
                            nc.vector.tensor_add(recM[:], psD[:], corrM[:])
                            nc.vector.reciprocal(recM[:], recM[:])
                            nc.vector.tensor_scalar_mul(recM[:], recM[:],
                                                        1.0 / OSC)
                            nc.sync.dma_start(rec_d[0:G2], recM[:])
                            nc.vector.memset(rtail[:], 0.0)
                            nc.sync.dma_start(rec_d[G2:GP], rtail[:])
                            nc.sync.dma_start(
                                recrep[:],
                                rec_d.rearrange("(a b) -> a b", b=TT)
                                .unsqueeze(0).broadcast_to([128, 20, TT]))
                            # indicator k-tiles queue behind the rec writes
                            # on SP: the rec write's wait blocks SP.SEQ, so
                            # this big transfer can't jump the gather/wo1
                            nc.sync.dma_start(dbuf[:, 4:6, :], ind_d[:])

                    # ---- e-field fill: dbuf = fp8(16 * X' (*) Y) ----
                    # DVE: fused 3-D STT per (chunk, n4) straight to fp8.
                    # Pool can't run STT (walrus), so its chunks are a
                    # tensor_tensor mult into etmp + an Act fp8 quant.
                    def estt(c):
                        for n4 in range(4):
                            nc.vector.scalar_tensor_tensor(
                                dbuf[:, n4, c * 500:(c + 1) * 500]
                                .rearrange("p (a c) -> p a c", c=100),
                                XP[:, n4, c * 5:(c + 1) * 5].unsqueeze(-1)
                                .broadcast_to([128, 5, 100]),
                                DSC,
                                Ye[:, n4, :].unsqueeze(1)
                                .broadcast_to([128, 5, 100]),
                                op0=ALU.mult, op1=ALU.mult)

                    with tc.tile_pool(name="etp", bufs=1) as etp:
                        etmps = {}

                        def emult(c):
                            etmp = etp.tile([128, 4, 500], F32, tag="etg",
                                            bufs=2, name=f"etmp{c}")
                            etmps[c] = etmp
                            for n4 in range(4):
                                nc.gpsimd.tensor_mul(
                                    etmp[:, n4, :]
                                    .rearrange("p (a c) -> p a c", c=100),
                                    XP[:, n4, c * 5:(c + 1) * 5]
                                    .unsqueeze(-1)
                                    .broadcast_to([128, 5, 100]),
                                    Ye[:, n4, :].unsqueeze(1)
                                    .broadcast_to([128, 5, 100]))

                        def equant(c):
                            nc.scalar.activation(
                                dbuf[:, 0:4, c * 500:(c + 1) * 500],
                                etmps.pop(c)[:], AF.Copy, scale=DSC)

                        for c in range(13):
                            estt(c)
                        emult(13)
                        emult(14)

                        # ---- wo1 ----
                        with tc.tile_pool(name="ops1", bufs=4,
                                          space="PSUM") as ops1:
                            for m in range(16):
                                ps = ops1.tile([128, NE], F32, tag="mmh")
                                for c in range(8):
                                    nc.tensor.matmul(
                                        ps[:],
                                        wo1[:, c, m * 128:(m + 1) * 128],
                                        selT[:, c, :], start=(c == 0),
                                        stop=(c == 7))
                                nc.scalar.activation(hT[:, m, :], ps[:],
                                                     AF.Gelu,
                                                     bias=bo1[:, m:m + 1])
                                if m % 2 == 1 and m <= 13:
                                    c0 = 13 + (m - 1) // 2
                                    equant(c0)
                                    if c0 + 2 < 20:
                                        emult(c0 + 2)

                    # ---- wo2 + attn, interleaved per dim-chunk m ----
                    with tc.tile_pool(name="oops", bufs=1,
                                      space="PSUM") as oops, \
                         tc.tile_pool(name="oo", bufs=5) as oo, \
                         tc.tile_pool(name="pjm", bufs=2) as pjm:
                        ops2 = opst = w2ps = oops
                        for m in range(4):
                            msl = slice(m * 128, (m + 1) * 128)
                            psW = ops2.tile([128, NE], F32, tag="mmp",
                                            bufs=1, name="psW")
                            for k in range(16):
                                nc.tensor.matmul(
                                    psW[:], wo2[:, k, msl],
                                    hT[:, k, :], start=(k == 0),
                                    stop=(k == 15))
                            projTm = pjm.tile([128, NE], BF16, tag="pjT")
                            nc.vector.tensor_scalar(
                                projTm[:], psW[:], bo2[:, m:m + 1], 0.0,
                                op0=ALU.add, op1=ALU.add,
                                accum_out=p0a[:, m:m + 1])
                            nc.vector.tensor_scalar_mul(
                                p0s[:, m:m + 1], p0a[:, m:m + 1], OSC)
                            for n4 in range(4):
                                pstp = opst.tile([128, 128], BF16,
                                                 tag="ptb", bufs=1,
                                                 name="pstp")
                                nc.tensor.transpose(
                                    pstp[:],
                                    projTm[:, n4 * 128:(n4 + 1) * 128],
                                    ident_bf[:])
                                nc.scalar.activation(
                                    projf8[:, n4, msl], pstp[:],
                                    AF.Copy, scale=PSC)
                            # W2 = Y'^T proj for this dim-chunk, hi/lo fp8
                            psW2 = w2ps.tile([100, 128], F32, tag="w2",
                                             bufs=1, name="psW2")
                            for k in range(4):
                                nc.tensor.matmul(
                                    psW2[:], YP8[:, k, :],
                                    projf8[:, k, msl],
                                    start=(k == 0), stop=(k == 3))
                            nc.scalar.activation(
                                projf8[0:100, 4, msl], psW2[:],
                                AF.Copy, scale=1.0 / 32.0)
                            tW2 = pjm.tile([100, 128], F32, tag="tw2")
                            nc.vector.scalar_tensor_tensor(
                                tW2[:], projf8[0:100, 4, msl], -32.0,
                                psW2[:], op0=ALU.mult, op1=ALU.add)
                            nc.vector.tensor_scalar(
                                projf8[0:100, 5, msl], tW2[:], 0.125, None,
                                op0=ALU.mult)

                            ngq, nper = 10, 2
                            for gq in range(ngq):
                                ot = oo.tile([128, nper, TT], BF16,
                                             tag="ot", name="ot")
                                for g2 in range(nper):
                                    gt = gq * nper + g2
                                    pso = oops.tile([128, TT], F32, tag="mmo",
                                                    bufs=5, name="pso")
                                    for c in range(3):
                                        nc.tensor.matmul(
                                            pso[:],
                                            projf8[:, 2 * c:2 * c + 2, msl],
                                            dbuf[:, 2 * c:2 * c + 2,
                                                 gt * TT:(gt + 1) * TT],
                                            start=(c == 0), stop=(c == 2),
                                            perf_mode=DR)
                                    # DVE fused STT takes 12 of 20; Pool
                                    # can't read PSUM, so its 8 tiles get an
                                    # Act Identity(psum+P0) drain first
                                    if gt % 5 < 3:
                                        nc.vector.scalar_tensor_tensor(
                                            ot[:, g2, :], pso[:],
                                            p0s[:, m:m + 1],
                                            recrep[:, gt, :],
                                            op0=ALU.add, op1=ALU.mult)
                                    else:
                                        ptm = oo.tile([128, TT], F32,
                                                      tag="ptm", bufs=2,
                                                      name="ptm")
                                        nc.scalar.activation(
                                            ptm[:], pso[:], AF.Identity,
                                            bias=p0s[:, m:m + 1])
                                        nc.gpsimd.tensor_mul(
                                            ot[:, g2, :], ptm[:],
                                            recrep[:, gt, :])
                                dq = nc.sync if gq % 2 == 0 else nc.scalar
                                dq.dma_start(
                                    out_d[msl, gq * nper * TT:
                                          (gq + 1) * nper * TT], ot[:])
    nc.compile()
    return nc


_NC_CACHE = {}


def _host_inputs(inputs):
    """Replicated host-side tensor prep (layout shuffles only)."""
    f32 = np.float32
    bf = ml_dtypes.bfloat16
    f8 = ml_dtypes.float8_e4m3fn
    W1 = np.asarray(inputs["W1"], f32)
    W2 = np.asarray(inputs["W2"], f32)
    Wc = np.asarray(inputs["Wc"], f32)
    We1 = np.asarray(inputs["We1"], f32)
    We2 = np.asarray(inputs["We2"], f32)
    Wo1 = np.asarray(inputs["Wo1"], f32)
    Wo2 = np.asarray(inputs["Wo2"], f32)
    b1 = np.asarray(inputs["b1"], f32); b2 = np.asarray(inputs["b2"], f32)
    bc = np.asarray(inputs["bc"], f32); be1 = np.asarray(inputs["be1"], f32)
    be2 = np.asarray(inputs["be2"], f32)
    bo1 = np.asarray(inputs["bo1"], f32); bo2 = np.asarray(inputs["bo2"], f32)

    def kchunk(w, nk):   # [K, N] -> [128, nk, N]
        return np.ascontiguousarray(
            w.reshape(nk, 128, w.shape[1]).transpose(1, 0, 2))

    def bvec(b, ncol):   # [N] -> [128, ncol]
        return np.ascontiguousarray(b.reshape(ncol, 128).T)

    # fold coords GEMM: fi@We1 = df@(We1[2:] + Wc@We1[:2]) + (be1 + bc@We1[:2])
    We1_64 = We1.astype(np.float64)
    we1_eff = (We1_64[2:] + Wc.astype(np.float64) @ We1_64[:2]).astype(f32)
    be1_eff = (be1.astype(np.float64)
               + bc.astype(np.float64) @ We1_64[:2]).astype(f32)

    We2_64 = We2.astype(np.float64)
    Aq = (We2_64 @ We2_64.T).astype(f32)               # [512, 512]
    vq = (2.0 * (We2_64 @ be2.astype(np.float64))).astype(f32)  # [512]

    # reorder We2 columns: [states (cols 2..1025) | px, py]; drop col 1026
    perm = np.concatenate([np.arange(2, 2 + E), np.array([0, 1])])
    we2r = np.ascontiguousarray(We2[:, perm])          # [512, 1026]
    be2r = np.zeros((9, 128), f32)
    be2r.ravel()[:E] = be2[2:2 + E]
    be2r[8, 0:2] = be2[0:2]
    be2r = np.ascontiguousarray(be2r.T)                # [128, 9]

    g = np.linspace(-1.0, 1.0, 100, dtype=np.float64)
    gv = np.ascontiguousarray(
        np.broadcast_to(g.astype(f32), (128, 100)))
    corrM = (1e-8 * np.exp(10.0 * (g[:, None] ** 2 + g[None, :] ** 2))
             ).astype(f32)

    # iy-indicator k-tiles for the W2 hi/lo rows of the attn GEMM
    iy = np.arange(GP) % 100
    ind = np.zeros((128, 2, GP), f8)
    eye = (np.arange(128)[:, None] == iy[None, :])
    ind[:, 0, :] = np.where(eye, np.float32(32.0), 0).astype(f8)
    ind[:, 1, :] = np.where(eye, np.float32(8.0), 0).astype(f8)

    return {
        "aq": kchunk(Aq, 4), "vq": bvec(vq, 4),
        "w1": kchunk(W1, 4), "w2": kchunk(W2, 4),
        "we1": kchunk(we1_eff, 4),
        "we2r": kchunk(we2r, 4).astype(bf),
        "wo1": kchunk(Wo1, 8).astype(bf),
        "wo2": kchunk(Wo2, 16).astype(bf),
        "b1": bvec(b1, 4), "b2": bvec(b2, 4),
        "be1": bvec(be1_eff, 4), "be2r": be2r,
        "bo1": bvec(bo1, 16), "bo2": bvec(bo2, 4),
        "gv": gv, "corrM": corrM, "ind": ind,
    }


def kernel(**inputs):
    if CHAIN not in _NC_CACHE:
        _NC_CACHE[CHAIN] = _build(CHAIN)
    nc = _NC_CACHE[CHAIN]
    shared = _host_inputs(inputs)
    x = np.asarray(inputs["x"], np.float32)
    in_maps = []
    for b in range(B):
        m = dict(shared)
        m["xT"] = np.ascontiguousarray(x[b].T)
        in_maps.append(m)
    res = run_bass_kernel_spmd(nc, in_maps, core_ids=list(range(B)))
    return np.stack([np.asarray(r["out"]).astype(np.float32)[:, :G2].T
                     for r in res.results])
